# revision 1
# baseline (speedup 1.0000x reference)
"""AFT-Full kernel for Trainium2, 8 NeuronCores.

Sharding: x [B=8, H=96, W=96, C=512] is split along H (dim 1) into 8 shards
of [8, 12, 96, 512].  Every step of the computation (q/k/v projections,
max over batch, the exp_w_bias matmuls over W, output projection) is local
to an H-slice, so there are no collectives at all.

I/O strategy (the kernel is HBM-bound): both the input and the output
live in HBM as bf16, halving DMA bytes vs f32.  The host pre-transposes
x into c-major layout [h, c, ch, pos] (pos = b*96 + w) so the kernel
needs NO on-device transposes of x — xT tiles are DMAed directly with
6 KB-contiguous per-partition descriptors.  The output is written
bf16 in [h, pos, c] layout and un-permuted / upcast on the host.

Software pipelining: each engine executes its instruction stream
IN ORDER, so the emission order is the schedule.  Iteration k emits
    PE : qkv pairs 0-1 (k) | num/den (k-1) | qkv pairs 2-3 (k) |
         yT transposes (k-1) | out projection M=128 x6 (k-1)
    DVE: den/rden/y (k-1) | v x4 (k) | yT copies (k-1) | o-copy (k-1) |
         emx/remx/eks/teq-add (k)
    ACT: exp(-q)/exp(k) x8 (k) | o-copies (k-1)
    GPS: max tree (k) | teq mul (k) | eks*v (k)
so the PE never waits a full cross-engine round trip: the element-wise
tail of iteration k-1 overlaps the qkv matmuls of iteration k.
"""

import sys

if "/opt/trn_rl_repo" not in sys.path:
    sys.path.insert(0, "/opt/trn_rl_repo")

import numpy as np
import ml_dtypes
from contextlib import ExitStack

import concourse.bass as bass
import concourse.bacc as bacc
import concourse.tile as tile
from concourse import masks, mybir
from concourse.bass_utils import run_bass_kernel_spmd

F32 = mybir.dt.float32
I32 = mybir.dt.int32
BF16 = mybir.dt.bfloat16
AF = mybir.ActivationFunctionType
BF16NP = ml_dtypes.bfloat16

B = 8          # batch
S = 96         # H = W = 96
C = 512        # input channels
D = 64         # hidden
HL = 12        # h rows per core
NCORES = 8
P = 128        # partitions
NPOS = B * S   # 768 positions per h
BLOB_F = 1504  # packed weight blob columns

_NC_CACHE = {}


def build_kernel():
    nc = bacc.Bacc()

    xt_d = nc.declare_dram_parameter("xt", [HL, P, 4, NPOS], BF16,
                                     isOutput=False)
    wblob_d = nc.declare_dram_parameter("wblob", [P, BLOB_F], BF16,
                                        isOutput=False)
    out_d = nc.declare_dram_parameter("out", [HL, NPOS, C], BF16,
                                      isOutput=True)

    with tile.TileContext(nc) as tc, ExitStack() as ctx:
        singles = ctx.enter_context(tc.tile_pool(name="singles", bufs=1))

        # ---------------- setup ----------------
        #   cols 0:768      wqkvT   [128, 4(chunk), 192]  (q|k|v columns)
        #   cols 768:1280   owT+b   [65, 512] (row 64 = out_b)
        #   cols 1280:1376  ewbT    [96, 96]  exp(w_bias)^T
        #   cols 1376:1440  eqbB    [96, 64]  exp(-wq_b) row-replicated
        #   cols 1440:1504  vbB     [96, 64]  wv_b row-replicated
        ident = singles.tile([P, P], BF16)
        masks.make_identity(nc, ident[:])

        # PSUM (bank-aligned tiles): qkv/yt 2x2 + nd 1x2 + o 2x1 = 8 banks
        qkv_ps = ctx.enter_context(tc.tile_pool(name="qkv", bufs=2, space="PSUM"))
        nd_ps = ctx.enter_context(tc.tile_pool(name="nd", bufs=1, space="PSUM"))
        o_ps = ctx.enter_context(tc.tile_pool(name="ops", bufs=2, space="PSUM"))

        blob = singles.tile([P, BLOB_F], BF16)
        warm = singles.tile([1, 8], BF16)
        zeros = singles.tile([P, C], BF16)
        nc.gpsimd.memset(warm[:], 0.0)
        nc.scalar.activation(warm[:], warm[:], AF.Exp)
        nc.vector.memset(zeros[:], 0.0)
        nc.sync.dma_start(blob[:], wblob_d[:, :])
        wqkv = blob[:, 0:768].rearrange("p (ch x) -> p ch x", ch=4)
        ow = blob[0:D + 1, 768:1280]
        ewbT = blob[0:S, 1280:1376]
        eqbB = blob[0:S, 1376:1440]
        vbB = blob[0:S, 1440:1504]

        # ---------------- main pools ----------------
        xT_pool = ctx.enter_context(tc.tile_pool(name="xT", bufs=3))
        vsb_pool = ctx.enter_context(tc.tile_pool(name="vsb", bufs=3))
        ek_pool = ctx.enter_context(tc.tile_pool(name="ek", bufs=3))
        eq_pool = ctx.enter_context(tc.tile_pool(name="eq", bufs=3))
        small_pool = ctx.enter_context(tc.tile_pool(name="small", bufs=2))
        eks_pool = ctx.enter_context(tc.tile_pool(name="eks", bufs=3))
        teq_pool = ctx.enter_context(tc.tile_pool(name="teq", bufs=3))
        den2_pool = ctx.enter_context(tc.tile_pool(name="den2", bufs=2))
        y_pool = ctx.enter_context(tc.tile_pool(name="y", bufs=2))
        yT_pool = ctx.enter_context(tc.tile_pool(name="yT", bufs=2))
        osb_pool = ctx.enter_context(tc.tile_pool(name="osb", bufs=3))

        # warm the PE HAM during the initial DMA ramp (results unused);
        # depends only on memset tiles so it overlaps the blob/x DMAs
        warm_ps = nd_ps.tile([S, B, 2 * D], F32, name="warmps", tag="ndp")
        for i in range(10):
            nc.tensor.matmul(warm_ps[:, (i % 2) * 4:(i % 2) * 4 + 4, :],
                             ident[:, :S], zeros[:], start=True, stop=True)

        yT_tiles = [yT_pool.tile([D + 1, NPOS], BF16, tag=f"yt{i}",
                                 name=f"yt{i}") for i in range(2)]
        for t in yT_tiles:
            nc.vector.memset(t[D:D + 1, :], 1.0)

        # ---------------- software-pipelined main loop ----------------
        xT_tiles = {}
        st = {}   # per-h live tiles

        def emit_load(h):
            xT_tiles[h] = xT_pool.tile([P, 4, NPOS], BF16, name=f"xT{h}",
                                       tag="xT")
            nc.sync.dma_start(xT_tiles[h][:], xt_d[h])

        def emit_qkv_half(h, half):
            # 16 matmuls for batches 4*half..4*half+3 into one 2-bank tile
            xT = xT_tiles[h]
            qp = qkv_ps.tile([S, 4, 4 * D], F32, tag="qp",
                             name=f"qp{h}_{half}")
            for sub in range(4):
                b = half * 4 + sub
                for ch in range(4):
                    nc.tensor.matmul(
                        qp[:, sub, 0:3 * D],
                        xT[:, ch, b * S:(b + 1) * S],
                        wqkv[:, ch, :],
                        start=(ch == 0),
                        stop=(ch == 3),
                    )
            return qp

        def emit_qkv_post(h, half, qp):
            # ACT: exps; DVE: v  (consume the half's PSUM tile)
            bsl = slice(half * 4, half * 4 + 4)
            nc.scalar.activation(st[h, "eq"][:, bsl, :], qp[:, :, 0:D],
                                 AF.Exp, scale=-1.0)
            nc.scalar.activation(st[h, "ek"][:, bsl, :], qp[:, :, D:2 * D],
                                 AF.Exp)
            nc.vector.tensor_add(
                st[h, "v"][:, bsl, :], qp[:, :, 2 * D:3 * D],
                vbB[:, :].rearrange("p (o d) -> p o d", o=1).broadcast_to([S, 4, D]))

        for k in range(HL + 1):
            if k == 0:
                emit_load(0)
                emit_load(1)
            if k + 2 < HL:
                emit_load(k + 2)

            if k < HL:
                st[k, "eq"] = eq_pool.tile([S, B, D], BF16, tag="eq", name=f"eq{k}")
                st[k, "ek"] = ek_pool.tile([S, B, D], BF16, tag="ek", name=f"ek{k}")
                st[k, "v"] = vsb_pool.tile([S, B, D], BF16, tag="v", name=f"v{k}")
                # PE: qkv batches 0-3 ; ACT/DVE evacuate
                qp = emit_qkv_half(k, 0)
                emit_qkv_post(k, 0, qp)
                # partial max over batches 0-3 as soon as half 0 lands
                mxA = small_pool.tile([S, 2, D], BF16, tag="mxA")
                nc.vector.tensor_max(mxA[:], st[k, "ek"][:, 0:2, :],
                                     st[k, "ek"][:, 2:4, :])
                st[k, "mxA"] = mxA

            if k >= 1:
                j = k - 1
                # PE: num/den matmuls for j (eks[j] was finished last iter)
                ndp = nd_ps.tile([S, B, 2 * D], F32, tag="ndp", name=f"ndp{j}")
                for half in range(2):
                    nc.tensor.matmul(ndp[:, half * 4:half * 4 + 4, :],
                                     ewbT[:],
                                     st[j, "eks"][:, half * 4:half * 4 + 4, :],
                                     start=True, stop=True)
                # DVE: y = num * recip(den * teq)
                den2 = den2_pool.tile([S, B, D], F32, tag="den2", name=f"den{j}")
                rden = den2_pool.tile([S, B, D], F32, tag="rden",
                                      name=f"rden{j}")
                y_sb = y_pool.tile([S, B, D], BF16, tag="y", name=f"y{j}")
                nc.vector.scalar_tensor_tensor(
                    den2[:], st[j, "teq"][:], 1.0, ndp[:, :, D:2 * D],
                    op0=mybir.AluOpType.add, op1=mybir.AluOpType.mult)
                nc.vector.reciprocal_approx_fast(rden[:], den2[:])
                nc.vector.tensor_mul(y_sb[:], ndp[:, :, 0:D], rden[:])

            if k < HL:
                # PE: qkv batches 4-7
                qp = emit_qkv_half(k, 1)
                emit_qkv_post(k, 1, qp)

            if k >= 1:
                j = k - 1
                # PE: transpose y -> yT [65, 768] (row 64 = ones); DVE copies
                yT = yT_tiles[j % 2]
                for half in range(2):
                    ytp_full = qkv_ps.tile([D, 1024], BF16, tag="qp")
                    ytp = ytp_full[:, 0:384]
                    for bb in range(4):
                        b = half * 4 + bb
                        nc.tensor.transpose(ytp[:, bb * S:(bb + 1) * S],
                                            y_sb[:, b, :], ident[:S, :S])
                    nc.scalar.copy(
                        yT[0:D, half * 384:(half + 1) * 384], ytp[:])

                # PE: output projection, M=128 pos-major; ACT/DVE copies
                o_sb = osb_pool.tile([P, 6, C], BF16, tag="osb", name=f"osb{j}")
                ops = []
                for t in range(6):
                    op = o_ps.tile([P, C], F32, tag="op")
                    nc.tensor.matmul(op[:], yT[:, t * P:(t + 1) * P], ow[:],
                                     start=True, stop=True)
                    ops.append(op)
                for t in range(6):
                    if t in (0, 3):
                        nc.vector.tensor_copy(o_sb[:, t, :], ops[t][:])
                    else:
                        nc.scalar.copy(o_sb[:, t, :], ops[t][:])
                dst = out_d[j].rearrange("(t p) c -> p t c", p=P)
                nc.sync.dma_start(dst, o_sb[:])

            if k < HL:
                # element-wise tail for k: stabilize + eks (DVE, short chain)
                # teq on GPS (not latency-critical until k+1)
                ek_raw = st[k, "ek"]
                mxB = small_pool.tile([S, 2, D], BF16, tag="mxB")
                mx2 = small_pool.tile([S, 2, D], BF16, tag="mx2")
                emx = small_pool.tile([S, D], F32, tag="emx")
                remx_f = small_pool.tile([S, D], F32, tag="remxf")
                remx = small_pool.tile([S, D], BF16, tag="remx")
                nc.vector.tensor_max(mxB[:], ek_raw[:, 4:6, :],
                                     ek_raw[:, 6:8, :])
                nc.vector.tensor_max(mx2[:], st[k, "mxA"][:], mxB[:])
                nc.vector.tensor_max(
                    emx[:], mx2[:, 0:1, :].rearrange("p o d -> p (o d)"),
                    mx2[:, 1:2, :].rearrange("p o d -> p (o d)"))
                nc.vector.reciprocal_approx_fast(remx_f[:], emx[:])
                nc.vector.tensor_copy(remx[:], remx_f[:])

                eks = eks_pool.tile([S, B, 2 * D], BF16, tag="eks", name=f"eks{k}")
                st[k, "eks"] = eks
                nc.vector.tensor_mul(
                    eks[:, :, D:2 * D], ek_raw[:],
                    remx[:, :].rearrange("p (o d) -> p o d", o=1).broadcast_to([S, B, D]))
                nc.vector.tensor_mul(eks[:, :, 0:D], eks[:, :, D:2 * D],
                                     st[k, "v"][:])

                teq = teq_pool.tile([S, B, D], BF16, tag="teq", name=f"teq{k}")
                nc.vector.tensor_mul(
                    teq[:], st[k, "eq"][:],
                    eqbB[:, :].rearrange("p (o d) -> p o d", o=1).broadcast_to([S, B, D]))
                st[k, "teq"] = teq

            if k >= 1:
                # drop references so pools can recycle
                for key in ("eq", "ek", "v", "eks", "teq", "mxA"):
                    st.pop((k - 1, key), None)
                xT_tiles.pop(k - 1, None)

    if not nc.is_finalized():
        nc.finalize()
    return nc


def _make_blob(wq_w, wq_b, wk_w, wk_b, wv_w, wv_b, out_w, out_b, w_bias_table):
    blob = np.zeros((P, BLOB_F), dtype=np.float32)
    for j, w in enumerate([wq_w, wk_w, wv_w]):       # wqkvT [128, 4, 192]
        for ch in range(4):
            # blob[p, ch*192 + j*64 + d] = w[d, ch*128 + p]
            blob[:, ch * 192 + j * D:(ch * 192 + (j + 1) * D)] = \
                w[:, ch * P:(ch + 1) * P].T
    blob[0:D, 768:1280] = np.asarray(out_w).T        # owT
    blob[D, 768:1280] = out_b
    blob[0:S, 1280:1376] = np.exp(np.asarray(w_bias_table)).T
    # wk_b cancels exactly in exp(k - max_b k); wq_b folded via exp(-wq_b),
    # wv_b added to v after the projection.
    blob[0:S, 1376:1440] = np.exp(-np.asarray(wq_b))[None, :]
    blob[0:S, 1440:1504] = np.asarray(wv_b)[None, :]
    return blob


def _build_in_maps(x, wq_w, wq_b, wk_w, wk_b, wv_w, wv_b, out_w, out_b,
                   w_bias_table):
    blob = _make_blob(wq_w, wq_b, wk_w, wk_b, wv_w, wv_b, out_w, out_b,
                      w_bias_table).astype(BF16NP)
    xbf = np.asarray(x).astype(BF16NP)               # [8, 96, 96, 512]
    in_maps = []
    for i in range(NCORES):
        # [B, HL, S, C] -> [HL, C, B, S] -> [HL, 4, 128, B*S]
        # -> [HL, 128, 4, B*S]  (c-within-chunk on partitions)
        xs = xbf[:, i * HL:(i + 1) * HL].transpose(1, 3, 0, 2)
        xs = xs.reshape(HL, 4, P, NPOS).transpose(0, 2, 1, 3)
        in_maps.append({
            "wblob": blob,
            "xt": np.ascontiguousarray(xs),
        })
    return in_maps


def kernel(x, wq_w, wq_b, wk_w, wk_b, wv_w, wv_b, out_w, out_b, w_bias_table):
    if "nc" not in _NC_CACHE:
        _NC_CACHE["nc"] = build_kernel()
    nc = _NC_CACHE["nc"]

    in_maps = _build_in_maps(x, wq_w, wq_b, wk_w, wk_b, wv_w, wv_b,
                             out_w, out_b, w_bias_table)
    res = run_bass_kernel_spmd(nc, in_maps, list(range(NCORES)))
    # per-core out: [HL, NPOS, C] bf16, pos = b*96+w -> [B, HL, S, C]
    outs = [np.asarray(res.results[i]["out"]).reshape(HL, B, S, C)
            .transpose(1, 0, 2, 3) for i in range(NCORES)]
    return np.concatenate(outs, axis=1).astype(np.float32)

